# revision 17
# baseline (speedup 1.0000x reference)
"""Trainium2 Bass kernel for a 2-layer LSTM decoder VAE head.

Strategy: 8-way tensor parallelism over the hidden dim (H=1024 -> 128 rows per
core); all state kept transposed ([feature, batch]) so no transposes are ever
needed; the output MLP is replicated on every core (cheaper than an AllReduce
of its tiny result).  Per step each core exchanges its h1/h2 chunks with the
other cores via AllGather.

The batch B=256 is split into two independent 128-wide chains whose step loops
are interleaved: while one chain waits for its AllGather to land, the PE runs
the other chain's matmuls.

v2 changes vs the first working kernel:
  - all 4 gates of a layer accumulate into ONE psum bank [128, 4, NB] with a
    single start=True on the first matmul (hardware has_written semantics
    clear the whole 2KB zero-region), so the nonlinearity is 2 ACT
    instructions (sigmoid over the i,f,o 3*NB slab + tanh over g) instead
    of 4+ with per-gate bias.
  - gate biases: layer0's ride an augmented ones-row on the Wih0 matmul
    (K=8 -> 9); b_proj rides an augmented column of W_proj; layer-1 /
    MLP biases are all-zero in this problem (asserted host-side, with a DVE
    fallback variant when nonzero).
  - MLP relu merged into one [128, 4, NB] bank + one ACT.
  - OUT is [8, seq, B] (contiguous per partition) and transposed on host.
  - emission order per step: whh1_A wih1_A | whh1_B wih1_B | whh0_A whh0_B
    MLP_A wih0_A MLP_B wih0_B -- every AllGather has ~6+us of independent
    matmul work queued between launch and first use.

Self-contained: only needs numpy + the concourse (Bass/Tile) runtime that is
preinstalled on the machine.
"""

import os
import numpy as np

B, SEQ, H, COORD = 256, 200, 1024, 8
LATS = (32, 64, 128)
TOT = sum(LATS)  # 224
N_CORES = 8
HC = H // N_CORES  # 128 rows per core
KT = H // 128      # 8 K tiles
NB = B // 2        # batch per chain

_CACHE = {}


def _mmdt():
    return os.environ.get("BASS_KERNEL_MMDT", "bf16")


def _build(seq, mmdt, has_bias1, has_bo1, has_bo2):
    import concourse.bass as bass
    import concourse.tile as tile
    from concourse import bacc, mybir

    f32 = mybir.dt.float32
    DT = {"fp32": mybir.dt.float32, "bf16": mybir.dt.bfloat16,
          "fp32r": mybir.dt.float32r}[mmdt]
    AF = mybir.ActivationFunctionType

    ND_WARM = int(os.environ.get("BASS_KERNEL_NDWARM", "48"))

    nc = bacc.Bacc("TRN2", target_bir_lowering=False, debug=False,
                   num_devices=N_CORES)

    def din(name, shape, dt=None):
        return nc.dram_tensor(name, list(shape), dt or f32,
                              kind="ExternalInput")

    # weights; gate order inside dim-1 is (i, f, o, g)
    whh0 = din("whh0", (128, 4, KT, 128), DT)
    wih1 = din("wih1", (128, 4, KT, 128), DT)
    whh1 = din("whh1", (128, 4, KT, 128), DT)
    wo1 = din("wo1", (128, 4, KT, 128), DT)
    wih0 = din("wih0", (9, 4, 128), DT)      # row 8 = gate bias (ones trick)
    wo2 = din("wo2", (128, 4, 8), DT)
    wproj = din("wproj", (128, 9, 2, 128))   # col TOT = b_proj (ones trick)
    zt = din("zt", (128, 2, B))
    bo2 = din("bo2", (8, 1))
    xinit = din("xinit", (9, NB), DT)        # zeros + ones row 8
    if has_bias1:
        bias1 = din("bias1", (128, 4, NB))   # broadcast of b_ih1+b_hh1
    if has_bo1:
        bo1b = din("bo1b", (128, 4, NB))     # broadcast of b_o1

    OUT = nc.dram_tensor("out", [seq, 8, B], f32, kind="ExternalOutput")

    def persist(name, shape, dtype=f32):
        return nc.alloc_sbuf_tensor(name, list(shape), dtype).ap()

    whh0_sb = persist("whh0_sb", [128, 4, KT, 128], DT)
    wih1_sb = persist("wih1_sb", [128, 4, KT, 128], DT)
    whh1_sb = persist("whh1_sb", [128, 4, KT, 128], DT)
    wo1_sb = persist("wo1_sb", [128, 4, KT, 128], DT)
    wih0_sb = persist("wih0_sb", [9, 4, 128], DT)
    wo2_sb = persist("wo2_sb", [128, 4, 8], DT)
    wproj_sb = persist("wproj_sb", [128, 9, 2, 128])
    zt_sb = persist("zt_sb", [128, 2, B])
    bo2_sb = persist("bo2_sb", [8, 1])
    if has_bias1:
        bias1_sb = persist("bias1_sb", [128, 4, NB])
    if has_bo1:
        bo1b_sb = persist("bo1b_sb", [128, 4, NB])

    class Chain:
        def __init__(self, name, b0):
            self.name = name
            self.b0 = b0                      # batch offset into OUT
            self.h1T = persist(f"h1T_{name}", [128, KT, NB], DT)
            self.h2T = persist(f"h2T_{name}", [128, KT, NB], DT)
            self.c1 = persist(f"c1_{name}", [128, NB])
            self.c2 = persist(f"c2_{name}", [128, NB])
            self.xT = persist(f"xT_{name}", [9, NB], DT)
            self.g0 = None                    # open gates0 psum bank
            self.g1 = None                    # open gates1 psum bank

    with tile.TileContext(nc) as tc:
        A = Chain("a", 0)
        Bc = Chain("b", NB)
        chains = (A, Bc)

        loads = [
            (whh0_sb, whh0), (wih1_sb, wih1), (whh1_sb, whh1), (wo1_sb, wo1),
            (wih0_sb, wih0), (wo2_sb, wo2), (wproj_sb, wproj), (zt_sb, zt),
            (bo2_sb, bo2),
        ]
        if has_bias1:
            loads.append((bias1_sb, bias1))
        if has_bo1:
            loads.append((bo1b_sb, bo1b))
        for dst, src in loads:
            nc.sync.dma_start(dst[:], src.ap())
        for ch in chains:
            nc.scalar.dma_start(ch.xT[:], xinit.ap())

        G_TAGS = ["g1a", "g1b", "g0a", "g0b", "ma", "mb"]

        with (
            tc.tile_pool(name="ps", bufs=1, space="PSUM") as ps,
            tc.tile_pool(name="psx", bufs=2, space="PSUM") as psx,
            tc.tile_pool(name="nl", bufs=2) as nl,
            tc.tile_pool(name="dram", bufs=2, space="DRAM") as dram,
        ):
            # ---- init: h0 full + own c0 chunk via W_proj (both chains) ----
            for m in range(9):
                pst = ps.tile([128, B], f32, name="ps_init",
                              tag=G_TAGS[m % 6])
                nc.tensor.matmul(pst[:], wproj_sb[:, m, 0, :], zt_sb[:, 0, :],
                                 start=True, stop=False)
                nc.tensor.matmul(pst[:], wproj_sb[:, m, 1, :], zt_sb[:, 1, :],
                                 start=False, stop=True)
                for ch in chains:
                    sl = pst[:, ch.b0:ch.b0 + NB]
                    if m < 8:
                        nc.vector.tensor_copy(ch.h1T[:, m, :], sl)
                        nc.scalar.activation(ch.h2T[:, m, :], sl, AF.Identity)
                    else:
                        nc.vector.tensor_copy(ch.c1[:, :], sl)
                        nc.scalar.activation(ch.c2[:, :], sl, AF.Identity)

            def warm_pe(nwarm):
                if nwarm <= 0:
                    return
                dps = psx.tile([128, NB], f32, name="ps_warm", tag="x")
                for i in range(nwarm):
                    nc.tensor.matmul(dps[:], whh1_sb[:, 0, i % KT, :],
                                     whh1_sb[:, 1, i % KT, :],
                                     start=True, stop=True)

            def lstm_nonlin(ch, G, c_sb, lay, bias_sb):
                """G: psum bank [128, 4, NB], gates (i, f, o, g)."""
                sfx = f"{ch.name}{lay}"
                if bias_sb is not None:
                    nc.vector.tensor_add(G[:, :, :], G[:, :, :], bias_sb[:])
                sig = nl.tile([128, 3, NB], f32, name="sig", tag=f"s_{sfx}")
                tan_g = nl.tile([128, NB], f32, name="tan_g", tag=f"tg_{sfx}")
                nc.scalar.activation(sig[:], G[:, 0:3, :], AF.Sigmoid)
                nc.scalar.activation(tan_g[:], G[:, 3, :], AF.Tanh)
                t_fc = nl.tile([128, NB], f32, name="t_fc", tag=f"fc_{sfx}")
                t_ig = nl.tile([128, NB], f32, name="t_ig", tag=f"ig_{sfx}")
                nc.vector.tensor_mul(t_fc[:], sig[:, 1, :], c_sb[:, :])
                nc.vector.tensor_mul(t_ig[:], sig[:, 0, :], tan_g[:])
                nc.vector.tensor_add(c_sb[:, :], t_fc[:], t_ig[:])
                tan_c = nl.tile([128, NB], f32, name="tan_c", tag=f"tc_{sfx}")
                nc.scalar.activation(tan_c[:], c_sb[:, :], AF.Tanh)
                hch = nl.tile([128, NB], DT, name="hch", tag=f"h_{sfx}")
                nc.vector.tensor_mul(hch[:], sig[:, 2, :], tan_c[:])
                return hch

            def gather(ch, hch, dest, lay):
                sfx = f"{ch.name}{lay}"
                inb = dram.tile([128, NB], DT, name="agin", tag=f"agi_{sfx}")
                outb = dram.tile([128 * N_CORES, NB], DT, name="agout",
                                 tag=f"ago_{sfx}", addr_space="Shared")
                nc.sync.dma_start(inb[:], hch[:])
                nc.gpsimd.collective_compute(
                    "AllGather", mybir.AluOpType.bypass,
                    replica_groups=[list(range(N_CORES))],
                    ins=[inb.opt()], outs=[outb.opt()],
                )
                for eng, k0, nk in ((nc.sync, 0, 4), (nc.scalar, 4, 4)):
                    eng.dma_start(
                        dest[:, k0:k0 + nk, :],
                        outb[k0 * 128:(k0 + nk) * 128, :].rearrange(
                            "(k p) n -> p k n", p=128))

            def emit_gates0_tail(ch):
                """wih0 into the open G0 bank, nonlin0, launch AG1."""
                G0 = ch.g0
                for g in range(4):
                    nc.tensor.matmul(G0[:, g, :], wih0_sb[:, g, :],
                                     ch.xT[:, :], start=False, stop=(g == 3))
                h1ch = lstm_nonlin(ch, G0, ch.c1, 0, None)
                gather(ch, h1ch, ch.h1T, 0)

            def emit_whh0(ch):
                """Whh0 @ h1 (ready work) -> open G0 bank [128, 4, NB]."""
                G0 = ps.tile([128, 4, NB], f32, name="ps_g0",
                             tag=f"g0{ch.name}")
                for k in range(KT):
                    for g in range(4):
                        nc.tensor.matmul(G0[:, g, :], whh0_sb[:, g, k, :],
                                         ch.h1T[:, k, :],
                                         start=(k == 0 and g == 0),
                                         stop=False)
                ch.g0 = G0

            def emit_front(ch):
                """Whh1 (ready work), then Wih1 (stalls on AG1), then the
                layer-1 nonlinearity and the h2 AllGather launch."""
                G1 = ps.tile([128, 4, NB], f32, name="ps_g1",
                             tag=f"g1{ch.name}")
                for k in range(KT):
                    for g in range(4):
                        nc.tensor.matmul(G1[:, g, :], whh1_sb[:, g, k, :],
                                         ch.h2T[:, k, :],
                                         start=(k == 0 and g == 0),
                                         stop=False)
                warm_pe(ND_WARM)
                for k in range(KT):
                    for g in range(4):
                        nc.tensor.matmul(G1[:, g, :], wih1_sb[:, g, k, :],
                                         ch.h1T[:, k, :],
                                         start=False,
                                         stop=(k == KT - 1 and g == 3))
                h2ch = lstm_nonlin(ch, G1, ch.c2, 1,
                                   bias1_sb[:] if has_bias1 else None)
                gather(ch, h2ch, ch.h2T, 1)

            def emit_mlp(ch, t, last):
                """MLP(t) (stalls on AG2), OUT write, then xT cast; if not
                last, wih0 completes gates0 and nonlin0 + AG1 launch."""
                warm_pe(ND_WARM)
                M = ps.tile([128, 4, NB], f32, name="ps_m",
                            tag=f"m{ch.name}")
                for k in range(KT):
                    for m in range(4):
                        nc.tensor.matmul(M[:, m, :], wo1_sb[:, m, k, :],
                                         ch.h2T[:, k, :],
                                         start=(k == 0 and m == 0),
                                         stop=(k == KT - 1 and m == 3))
                relu = nl.tile([128, 4, NB], DT, name="relu",
                               tag=f"relu_{ch.name}")
                if has_bo1:
                    nc.vector.tensor_add(M[:, :, :], M[:, :, :], bo1b_sb[:])
                nc.scalar.activation(relu[:], M[:], AF.Relu)
                ps_x = psx.tile([8, NB], f32, name="ps_x", tag="x")
                for k in range(4):
                    nc.tensor.matmul(ps_x[:], wo2_sb[:, k, :], relu[:, k, :],
                                     start=(k == 0), stop=(k == 3))
                if has_bo2:
                    nc.scalar.activation(ch.xT[:8, :], ps_x[:], AF.Identity,
                                         bias=bo2_sb[:, 0:1])
                else:
                    nc.vector.tensor_copy(ch.xT[:8, :], ps_x[:])
                xTf = nl.tile([8, NB], f32, name="xTf", tag=f"xf_{ch.name}")
                nc.scalar.activation(xTf[:], ps_x[:], AF.Identity,
                                     bias=bo2_sb[:, 0:1])
                nc.sync.dma_start(OUT.ap()[t][:, ch.b0:ch.b0 + NB], xTf[:])
                if not last:
                    emit_gates0_tail(ch)

            # ---- step 0 prologue: gates0(0) = Whh0@h0 + Wih0@[0;1] ----
            for ch in chains:
                emit_whh0(ch)
                emit_gates0_tail(ch)

            for t in range(seq):
                last = t == seq - 1
                for ch in chains:
                    emit_front(ch)
                if not last:
                    for ch in chains:
                        emit_whh0(ch)
                for ch in chains:
                    emit_mlp(ch, t, last)

    nc.compile()
    return nc


def _lhsT_tiles(W, rows, K):
    """W[rows] viewed as lhsT tiles: [128, MT, KTl, 128] with
    out[ki, mt, kt, mi] = W[rows[mt*128+mi], kt*128+ki]."""
    R = len(rows)
    MT = R // 128
    KTl = K // 128
    t = W[rows].reshape(MT, 128, KTl, 128)          # [mt, mi, kt, ki]
    return np.ascontiguousarray(t.transpose(3, 0, 2, 1)).astype(np.float32)


GATE_PERM = (0, 1, 3, 2)  # pytorch (i,f,g,o) -> kernel (i,f,o,g)


def _prep_inputs(inputs):
    import ml_dtypes
    np_dt = {"fp32": np.float32, "bf16": ml_dtypes.bfloat16,
             "fp32r": np.float32}[_mmdt()]
    f = lambda k: np.asarray(inputs[k], np.float32)
    W_proj, b_proj = f("W_proj"), f("b_proj")
    W_ih0, W_hh0 = f("W_ih0"), f("W_hh0")
    b_ih0, b_hh0 = f("b_ih0"), f("b_hh0")
    W_ih1, W_hh1 = f("W_ih1"), f("W_hh1")
    b_ih1, b_hh1 = f("b_ih1"), f("b_hh1")
    W_o1, b_o1 = f("W_o1"), f("b_o1")
    W_o2, b_o2 = f("W_o2"), f("b_o2")
    z = np.concatenate([f("z_primitive"), f("z_skill"), f("z_style")], axis=1)

    wo1 = _lhsT_tiles(W_o1, np.arange(512), H)
    wo2 = np.ascontiguousarray(
        W_o2.T.reshape(4, 128, 8).transpose(1, 0, 2)).astype(np.float32)
    bo2 = b_o2.reshape(8, 1).astype(np.float32)
    bias_g1 = b_ih1 + b_hh1
    has_bias1 = bool(np.any(bias_g1 != 0.0))
    has_bo1 = bool(np.any(b_o1 != 0.0))
    has_bo2 = bool(np.any(b_o2 != 0.0))

    ztp = np.zeros((256, B), np.float32)
    ztp[:TOT] = z.T
    ztp[TOT] = 1.0
    zt = np.ascontiguousarray(ztp.reshape(2, 128, B).transpose(1, 0, 2))
    Wp = np.zeros((2 * H, 256), np.float32)
    Wp[:, :TOT] = W_proj
    Wp[:, TOT] = b_proj
    bias_g0 = b_ih0 + b_hh0
    xinit = np.zeros((9, NB), np.float32)
    xinit[8] = 1.0

    in_maps = []
    for c in range(N_CORES):
        rows_g = np.concatenate(
            [GATE_PERM[g] * H + c * HC + np.arange(HC) for g in range(4)])
        rows_p = np.concatenate([np.arange(H), H + c * HC + np.arange(HC)])
        wih0 = np.zeros((9, 4, 128), np.float32)
        wih0[:8] = W_ih0[rows_g].reshape(4, 128, 8).transpose(2, 0, 1)
        wih0[8] = bias_g0[rows_g].reshape(4, 128)
        im = {
            "whh0": _lhsT_tiles(W_hh0, rows_g, H).astype(np_dt),
            "wih1": _lhsT_tiles(W_ih1, rows_g, H).astype(np_dt),
            "whh1": _lhsT_tiles(W_hh1, rows_g, H).astype(np_dt),
            "wo1": wo1.astype(np_dt),
            "wih0": wih0.astype(np_dt),
            "wo2": wo2.astype(np_dt),
            "wproj": _lhsT_tiles(Wp, rows_p, 256),
            "zt": zt,
            "bo2": bo2,
            "xinit": xinit.astype(np_dt),
        }
        if has_bias1:
            im["bias1"] = np.broadcast_to(
                bias_g1[rows_g].reshape(4, 128).T[:, :, None],
                (128, 4, NB)).astype(np.float32).copy()
        if has_bo1:
            im["bo1b"] = np.broadcast_to(
                b_o1.reshape(4, 128).T[:, :, None],
                (128, 4, NB)).astype(np.float32).copy()
        in_maps.append(im)
    return in_maps, has_bias1, has_bo1, has_bo2


def kernel(**inputs):
    from concourse.bass_utils import run_bass_kernel_spmd

    seq = int(os.environ.get("BASS_KERNEL_SEQ", SEQ))
    in_maps, has_bias1, has_bo1, has_bo2 = _prep_inputs(inputs)
    key = (seq, _mmdt(), has_bias1, has_bo1, has_bo2)
    if key not in _CACHE:
        _CACHE[key] = _build(seq, _mmdt(), has_bias1, has_bo1, has_bo2)
    nc = _CACHE[key]

    trace = os.environ.get("BASS_KERNEL_TRACE", "") == "1"
    kwargs = {}
    if trace:
        kwargs["trace"] = True
        kwargs["tmpdir"] = os.environ.get("BASS_KERNEL_TRACE_DIR") or None
    res = run_bass_kernel_spmd(nc, in_maps, core_ids=list(range(N_CORES)),
                               **kwargs)
    if trace:
        kernel.last_exec_time_ns = res.exec_time_ns
    out = res.results[0]["out"]          # [seq, 8, B]
    return np.ascontiguousarray(out.transpose(2, 0, 1)).astype(np.float32)


kernel.last_exec_time_ns = None


# revision 19
# speedup vs baseline: 1.0781x; 1.0781x over previous
"""Trainium2 Bass kernel for a 2-layer LSTM decoder VAE head.

Strategy: 8-way tensor parallelism over the hidden dim (H=1024 -> 128 rows per
core); all state kept transposed ([feature, batch]) so no transposes are ever
needed; the output MLP is replicated on every core (cheaper than an AllReduce
of its tiny result).  Per step each core exchanges its h1/h2 chunks with the
other cores via AllGather.

The batch B=256 is split into two independent 128-wide chains whose step loops
are interleaved: while one chain waits for its AllGather to land, the PE runs
the other chain's matmuls.

v2 changes vs the first working kernel:
  - all 4 gates of a layer accumulate into ONE psum bank [128, 4, NB] with a
    single start=True on the first matmul (hardware has_written semantics
    clear the whole 2KB zero-region), so the nonlinearity is 2 ACT
    instructions (sigmoid over the i,f,o 3*NB slab + tanh over g) instead
    of 4+ with per-gate bias.
  - gate biases: layer0's ride an augmented ones-row on the Wih0 matmul
    (K=8 -> 9); b_proj rides an augmented column of W_proj; layer-1 /
    MLP biases are all-zero in this problem (asserted host-side, with a DVE
    fallback variant when nonzero).
  - MLP relu merged into one [128, 4, NB] bank + one ACT.
  - OUT is [8, seq, B] (contiguous per partition) and transposed on host.
  - emission order per step: whh1_A wih1_A | whh1_B wih1_B | whh0_A whh0_B
    MLP_A wih0_A MLP_B wih0_B -- every AllGather has ~6+us of independent
    matmul work queued between launch and first use.

Self-contained: only needs numpy + the concourse (Bass/Tile) runtime that is
preinstalled on the machine.
"""

import os
import numpy as np

B, SEQ, H, COORD = 256, 200, 1024, 8
LATS = (32, 64, 128)
TOT = sum(LATS)  # 224
N_CORES = 8
HC = H // N_CORES  # 128 rows per core
KT = H // 128      # 8 K tiles
NB = B // 2        # batch per chain

_CACHE = {}


def _mmdt():
    return os.environ.get("BASS_KERNEL_MMDT", "bf16")


def _build(seq, mmdt, has_bias1, has_bo1, has_bo2):
    import concourse.bass as bass
    import concourse.tile as tile
    from concourse import bacc, mybir

    f32 = mybir.dt.float32
    DT = {"fp32": mybir.dt.float32, "bf16": mybir.dt.bfloat16,
          "fp32r": mybir.dt.float32r}[mmdt]
    AF = mybir.ActivationFunctionType

    ND_WARM = int(os.environ.get("BASS_KERNEL_NDWARM", "0"))

    nc = bacc.Bacc("TRN2", target_bir_lowering=False, debug=False,
                   num_devices=N_CORES)

    def din(name, shape, dt=None):
        return nc.dram_tensor(name, list(shape), dt or f32,
                              kind="ExternalInput")

    # weights; gate order inside dim-1 is (i, f, o, g)
    whh0 = din("whh0", (128, 4, KT, 128), DT)
    wih1 = din("wih1", (128, 4, KT, 128), DT)
    whh1 = din("whh1", (128, 4, KT, 128), DT)
    wo1 = din("wo1", (128, 4, KT, 128), DT)
    wih0 = din("wih0", (9, 4, 128), DT)      # row 8 = gate bias (ones trick)
    wo2 = din("wo2", (128, 4, 8), DT)
    wproj = din("wproj", (128, 9, 2, 128))   # col TOT = b_proj (ones trick)
    zt = din("zt", (128, 2, B))
    bo2 = din("bo2", (8, 1))
    xinit = din("xinit", (9, NB), DT)        # zeros + ones row 8
    if has_bias1:
        bias1 = din("bias1", (128, 4, NB))   # broadcast of b_ih1+b_hh1
    if has_bo1:
        bo1b = din("bo1b", (128, 4, NB))     # broadcast of b_o1

    OUT = nc.dram_tensor("out", [seq, 8, B], f32, kind="ExternalOutput")

    def persist(name, shape, dtype=f32):
        return nc.alloc_sbuf_tensor(name, list(shape), dtype).ap()

    whh0_sb = persist("whh0_sb", [128, 4, KT, 128], DT)
    wih1_sb = persist("wih1_sb", [128, 4, KT, 128], DT)
    whh1_sb = persist("whh1_sb", [128, 4, KT, 128], DT)
    wo1_sb = persist("wo1_sb", [128, 4, KT, 128], DT)
    wih0_sb = persist("wih0_sb", [9, 4, 128], DT)
    wo2_sb = persist("wo2_sb", [128, 4, 8], DT)
    wproj_sb = persist("wproj_sb", [128, 9, 2, 128])
    zt_sb = persist("zt_sb", [128, 2, B])
    bo2_sb = persist("bo2_sb", [8, 1])
    if has_bias1:
        bias1_sb = persist("bias1_sb", [128, 4, NB])
    if has_bo1:
        bo1b_sb = persist("bo1b_sb", [128, 4, NB])

    class Chain:
        def __init__(self, name, b0):
            self.name = name
            self.b0 = b0                      # batch offset into OUT
            self.h1T = persist(f"h1T_{name}", [128, KT, NB], DT)
            self.h2T = persist(f"h2T_{name}", [128, KT, NB], DT)
            self.c1 = persist(f"c1_{name}", [128, NB])
            self.c2 = persist(f"c2_{name}", [128, NB])
            self.xT = persist(f"xT_{name}", [9, NB], DT)
            self.g0 = None                    # open gates0 psum bank
            self.g1 = None                    # open gates1 psum bank

    with tile.TileContext(nc) as tc:
        A = Chain("a", 0)
        Bc = Chain("b", NB)
        chains = (A, Bc)

        loads = [
            (whh0_sb, whh0), (wih1_sb, wih1), (whh1_sb, whh1), (wo1_sb, wo1),
            (wih0_sb, wih0), (wo2_sb, wo2), (wproj_sb, wproj), (zt_sb, zt),
            (bo2_sb, bo2),
        ]
        if has_bias1:
            loads.append((bias1_sb, bias1))
        if has_bo1:
            loads.append((bo1b_sb, bo1b))
        for dst, src in loads:
            nc.sync.dma_start(dst[:], src.ap())
        for ch in chains:
            nc.scalar.dma_start(ch.xT[:], xinit.ap())

        G_TAGS = ["g1a", "g1b", "g0a", "g0b", "ma", "mb"]

        with (
            tc.tile_pool(name="ps", bufs=1, space="PSUM") as ps,
            tc.tile_pool(name="psx", bufs=2, space="PSUM") as psx,
            tc.tile_pool(name="nl", bufs=2) as nl,
            tc.tile_pool(name="dram", bufs=2, space="DRAM") as dram,
        ):
            # ---- init: h0 full + own c0 chunk via W_proj (both chains) ----
            for m in range(9):
                pst = ps.tile([128, B], f32, name="ps_init",
                              tag=G_TAGS[m % 6])
                nc.tensor.matmul(pst[:], wproj_sb[:, m, 0, :], zt_sb[:, 0, :],
                                 start=True, stop=False)
                nc.tensor.matmul(pst[:], wproj_sb[:, m, 1, :], zt_sb[:, 1, :],
                                 start=False, stop=True)
                for ch in chains:
                    sl = pst[:, ch.b0:ch.b0 + NB]
                    if m < 8:
                        nc.vector.tensor_copy(ch.h1T[:, m, :], sl)
                        nc.scalar.activation(ch.h2T[:, m, :], sl, AF.Identity)
                    else:
                        nc.vector.tensor_copy(ch.c1[:, :], sl)
                        nc.scalar.activation(ch.c2[:, :], sl, AF.Identity)

            def warm_pe(nwarm):
                if nwarm <= 0:
                    return
                dps = psx.tile([128, NB], f32, name="ps_warm", tag="x")
                for i in range(nwarm):
                    nc.tensor.matmul(dps[:], whh1_sb[:, 0, i % KT, :],
                                     whh1_sb[:, 1, i % KT, :],
                                     start=True, stop=True)

            def lstm_nonlin(ch, G, c_sb, lay, bias_sb):
                """G: psum bank [128, 4, NB], gates (i, f, o, g)."""
                sfx = f"{ch.name}{lay}"
                if bias_sb is not None:
                    nc.vector.tensor_add(G[:, :, :], G[:, :, :], bias_sb[:])
                sig = nl.tile([128, 3, NB], f32, name="sig", tag=f"s_{sfx}")
                tan_g = nl.tile([128, NB], f32, name="tan_g", tag=f"tg_{sfx}")
                nc.scalar.activation(sig[:], G[:, 0:3, :], AF.Sigmoid)
                nc.scalar.activation(tan_g[:], G[:, 3, :], AF.Tanh)
                t_fc = nl.tile([128, NB], f32, name="t_fc", tag=f"fc_{sfx}")
                t_ig = nl.tile([128, NB], f32, name="t_ig", tag=f"ig_{sfx}")
                nc.vector.tensor_mul(t_fc[:], sig[:, 1, :], c_sb[:, :])
                nc.vector.tensor_mul(t_ig[:], sig[:, 0, :], tan_g[:])
                nc.vector.tensor_add(c_sb[:, :], t_fc[:], t_ig[:])
                tan_c = nl.tile([128, NB], f32, name="tan_c", tag=f"tc_{sfx}")
                nc.scalar.activation(tan_c[:], c_sb[:, :], AF.Tanh)
                hch = nl.tile([128, NB], DT, name="hch", tag=f"h_{sfx}")
                nc.vector.tensor_mul(hch[:], sig[:, 2, :], tan_c[:])
                return hch

            def gather(ch, hch, dest, lay):
                sfx = f"{ch.name}{lay}"
                inb = dram.tile([128, NB], DT, name="agin", tag=f"agi_{sfx}")
                outb = dram.tile([128 * N_CORES, NB], DT, name="agout",
                                 tag=f"ago_{sfx}", addr_space="Shared")
                nc.scalar.dma_start(inb[:], hch[:])
                nc.gpsimd.collective_compute(
                    "AllGather", mybir.AluOpType.bypass,
                    replica_groups=[list(range(N_CORES))],
                    ins=[inb.opt()], outs=[outb.opt()],
                )
                for eng, k0, nk in ((nc.sync, 0, 2), (nc.scalar, 2, 2),
                                    (nc.sync, 4, 2), (nc.scalar, 6, 2)):
                    eng.dma_start(
                        dest[:, k0:k0 + nk, :],
                        outb[k0 * 128:(k0 + nk) * 128, :].rearrange(
                            "(k p) n -> p k n", p=128))

            def emit_gates0_tail(ch):
                """wih0 into the open G0 bank, nonlin0, launch AG1."""
                G0 = ch.g0
                for g in range(4):
                    nc.tensor.matmul(G0[:, g, :], wih0_sb[:, g, :],
                                     ch.xT[:, :], start=False, stop=(g == 3))
                h1ch = lstm_nonlin(ch, G0, ch.c1, 0, None)
                gather(ch, h1ch, ch.h1T, 0)

            def emit_whh0(ch):
                """Whh0 @ h1 (ready work) -> open G0 bank [128, 4, NB]."""
                G0 = ps.tile([128, 4, NB], f32, name="ps_g0",
                             tag=f"g0{ch.name}")
                for k in range(KT):
                    for g in range(4):
                        nc.tensor.matmul(G0[:, g, :], whh0_sb[:, g, k, :],
                                         ch.h1T[:, k, :],
                                         start=(k == 0 and g == 0),
                                         stop=False)
                ch.g0 = G0

            def emit_front(ch):
                """Whh1 (ready work), then Wih1 (stalls on AG1), then the
                layer-1 nonlinearity and the h2 AllGather launch."""
                G1 = ps.tile([128, 4, NB], f32, name="ps_g1",
                             tag=f"g1{ch.name}")
                for k in range(KT):
                    for g in range(4):
                        nc.tensor.matmul(G1[:, g, :], whh1_sb[:, g, k, :],
                                         ch.h2T[:, k, :],
                                         start=(k == 0 and g == 0),
                                         stop=False)
                warm_pe(ND_WARM)
                for k in range(KT):
                    for g in range(4):
                        nc.tensor.matmul(G1[:, g, :], wih1_sb[:, g, k, :],
                                         ch.h1T[:, k, :],
                                         start=False,
                                         stop=(k == KT - 1 and g == 3))
                h2ch = lstm_nonlin(ch, G1, ch.c2, 1,
                                   bias1_sb[:] if has_bias1 else None)
                gather(ch, h2ch, ch.h2T, 1)

            def emit_mlp(ch, t, last):
                """MLP(t) (stalls on AG2), OUT write, then xT cast; if not
                last, wih0 completes gates0 and nonlin0 + AG1 launch."""
                warm_pe(ND_WARM)
                M = ps.tile([128, 4, NB], f32, name="ps_m",
                            tag=f"m{ch.name}")
                for k in range(KT):
                    for m in range(4):
                        nc.tensor.matmul(M[:, m, :], wo1_sb[:, m, k, :],
                                         ch.h2T[:, k, :],
                                         start=(k == 0 and m == 0),
                                         stop=(k == KT - 1 and m == 3))
                relu = nl.tile([128, 4, NB], DT, name="relu",
                               tag=f"relu_{ch.name}")
                if has_bo1:
                    nc.vector.tensor_add(M[:, :, :], M[:, :, :], bo1b_sb[:])
                nc.scalar.activation(relu[:], M[:], AF.Relu)
                ps_x = psx.tile([8, NB], f32, name="ps_x", tag="x")
                for k in range(4):
                    nc.tensor.matmul(ps_x[:], wo2_sb[:, k, :], relu[:, k, :],
                                     start=(k == 0), stop=(k == 3))
                if has_bo2:
                    nc.scalar.activation(ch.xT[:8, :], ps_x[:], AF.Identity,
                                         bias=bo2_sb[:, 0:1])
                else:
                    nc.vector.tensor_copy(ch.xT[:8, :], ps_x[:])
                xTf = nl.tile([8, NB], f32, name="xTf", tag=f"xf_{ch.name}")
                nc.scalar.activation(xTf[:], ps_x[:], AF.Identity,
                                     bias=bo2_sb[:, 0:1])
                nc.sync.dma_start(OUT.ap()[t][:, ch.b0:ch.b0 + NB], xTf[:])
                if not last:
                    emit_gates0_tail(ch)

            # ---- step 0 prologue: gates0(0) = Whh0@h0 + Wih0@[0;1] ----
            for ch in chains:
                emit_whh0(ch)
                emit_gates0_tail(ch)

            for t in range(seq):
                last = t == seq - 1
                for ch in chains:
                    emit_front(ch)
                if not last:
                    for ch in chains:
                        emit_whh0(ch)
                for ch in chains:
                    emit_mlp(ch, t, last)

    nc.compile()
    return nc


def _lhsT_tiles(W, rows, K):
    """W[rows] viewed as lhsT tiles: [128, MT, KTl, 128] with
    out[ki, mt, kt, mi] = W[rows[mt*128+mi], kt*128+ki]."""
    R = len(rows)
    MT = R // 128
    KTl = K // 128
    t = W[rows].reshape(MT, 128, KTl, 128)          # [mt, mi, kt, ki]
    return np.ascontiguousarray(t.transpose(3, 0, 2, 1)).astype(np.float32)


GATE_PERM = (0, 1, 3, 2)  # pytorch (i,f,g,o) -> kernel (i,f,o,g)


def _prep_inputs(inputs):
    import ml_dtypes
    np_dt = {"fp32": np.float32, "bf16": ml_dtypes.bfloat16,
             "fp32r": np.float32}[_mmdt()]
    f = lambda k: np.asarray(inputs[k], np.float32)
    W_proj, b_proj = f("W_proj"), f("b_proj")
    W_ih0, W_hh0 = f("W_ih0"), f("W_hh0")
    b_ih0, b_hh0 = f("b_ih0"), f("b_hh0")
    W_ih1, W_hh1 = f("W_ih1"), f("W_hh1")
    b_ih1, b_hh1 = f("b_ih1"), f("b_hh1")
    W_o1, b_o1 = f("W_o1"), f("b_o1")
    W_o2, b_o2 = f("W_o2"), f("b_o2")
    z = np.concatenate([f("z_primitive"), f("z_skill"), f("z_style")], axis=1)

    wo1 = _lhsT_tiles(W_o1, np.arange(512), H)
    wo2 = np.ascontiguousarray(
        W_o2.T.reshape(4, 128, 8).transpose(1, 0, 2)).astype(np.float32)
    bo2 = b_o2.reshape(8, 1).astype(np.float32)
    bias_g1 = b_ih1 + b_hh1
    has_bias1 = bool(np.any(bias_g1 != 0.0))
    has_bo1 = bool(np.any(b_o1 != 0.0))
    has_bo2 = bool(np.any(b_o2 != 0.0))

    ztp = np.zeros((256, B), np.float32)
    ztp[:TOT] = z.T
    ztp[TOT] = 1.0
    zt = np.ascontiguousarray(ztp.reshape(2, 128, B).transpose(1, 0, 2))
    Wp = np.zeros((2 * H, 256), np.float32)
    Wp[:, :TOT] = W_proj
    Wp[:, TOT] = b_proj
    bias_g0 = b_ih0 + b_hh0
    xinit = np.zeros((9, NB), np.float32)
    xinit[8] = 1.0

    in_maps = []
    for c in range(N_CORES):
        rows_g = np.concatenate(
            [GATE_PERM[g] * H + c * HC + np.arange(HC) for g in range(4)])
        rows_p = np.concatenate([np.arange(H), H + c * HC + np.arange(HC)])
        wih0 = np.zeros((9, 4, 128), np.float32)
        wih0[:8] = W_ih0[rows_g].reshape(4, 128, 8).transpose(2, 0, 1)
        wih0[8] = bias_g0[rows_g].reshape(4, 128)
        im = {
            "whh0": _lhsT_tiles(W_hh0, rows_g, H).astype(np_dt),
            "wih1": _lhsT_tiles(W_ih1, rows_g, H).astype(np_dt),
            "whh1": _lhsT_tiles(W_hh1, rows_g, H).astype(np_dt),
            "wo1": wo1.astype(np_dt),
            "wih0": wih0.astype(np_dt),
            "wo2": wo2.astype(np_dt),
            "wproj": _lhsT_tiles(Wp, rows_p, 256),
            "zt": zt,
            "bo2": bo2,
            "xinit": xinit.astype(np_dt),
        }
        if has_bias1:
            im["bias1"] = np.broadcast_to(
                bias_g1[rows_g].reshape(4, 128).T[:, :, None],
                (128, 4, NB)).astype(np.float32).copy()
        if has_bo1:
            im["bo1b"] = np.broadcast_to(
                b_o1.reshape(4, 128).T[:, :, None],
                (128, 4, NB)).astype(np.float32).copy()
        in_maps.append(im)
    return in_maps, has_bias1, has_bo1, has_bo2


def kernel(**inputs):
    from concourse.bass_utils import run_bass_kernel_spmd

    seq = int(os.environ.get("BASS_KERNEL_SEQ", SEQ))
    in_maps, has_bias1, has_bo1, has_bo2 = _prep_inputs(inputs)
    key = (seq, _mmdt(), has_bias1, has_bo1, has_bo2)
    if key not in _CACHE:
        _CACHE[key] = _build(seq, _mmdt(), has_bias1, has_bo1, has_bo2)
    nc = _CACHE[key]

    trace = os.environ.get("BASS_KERNEL_TRACE", "") == "1"
    kwargs = {}
    if trace:
        kwargs["trace"] = True
        kwargs["tmpdir"] = os.environ.get("BASS_KERNEL_TRACE_DIR") or None
    res = run_bass_kernel_spmd(nc, in_maps, core_ids=list(range(N_CORES)),
                               **kwargs)
    if trace:
        kernel.last_exec_time_ns = res.exec_time_ns
    out = res.results[0]["out"]          # [seq, 8, B]
    return np.ascontiguousarray(out.transpose(2, 0, 1)).astype(np.float32)


kernel.last_exec_time_ns = None


# revision 20
# speedup vs baseline: 1.0915x; 1.0124x over previous
"""Trainium2 Bass kernel for a 2-layer LSTM decoder VAE head.

Strategy: 8-way tensor parallelism over the hidden dim (H=1024 -> 128 rows per
core); all state kept transposed ([feature, batch]) so no transposes are ever
needed; the output MLP is replicated on every core (cheaper than an AllReduce
of its tiny result).  Per step each core exchanges its h1/h2 chunks with the
other cores via AllGather.

The batch B=256 is split into two independent 128-wide chains whose step loops
are interleaved: while one chain waits for its AllGather to land, the PE runs
the other chain's matmuls.

v2 changes vs the first working kernel:
  - all 4 gates of a layer accumulate into ONE psum bank [128, 4, NB] with a
    single start=True on the first matmul (hardware has_written semantics
    clear the whole 2KB zero-region), so the nonlinearity is 2 ACT
    instructions (sigmoid over the i,f,o 3*NB slab + tanh over g) instead
    of 4+ with per-gate bias.
  - gate biases: layer0's ride an augmented ones-row on the Wih0 matmul
    (K=8 -> 9); b_proj rides an augmented column of W_proj; layer-1 /
    MLP biases are all-zero in this problem (asserted host-side, with a DVE
    fallback variant when nonzero).
  - MLP relu merged into one [128, 4, NB] bank + one ACT.
  - AllGather edge trimmed: pack DMA on the scalar HWDGE queue (no queueing
    behind the other chain's unpacks), unpack split 4-way sync/scalar in
    ascending k order so the consumer's k0 matmuls start as soon as the
    first quarter lands.

Self-contained: only needs numpy + the concourse (Bass/Tile) runtime that is
preinstalled on the machine.
"""

import os
import numpy as np

B, SEQ, H, COORD = 256, 200, 1024, 8
LATS = (32, 64, 128)
TOT = sum(LATS)  # 224
N_CORES = 8
HC = H // N_CORES  # 128 rows per core
KT = H // 128      # 8 K tiles
NB = B // 2        # batch per chain

_CACHE = {}


def _mmdt():
    return os.environ.get("BASS_KERNEL_MMDT", "bf16")


def _build(seq, mmdt, has_bias1, has_bo1, has_bo2):
    import concourse.bass as bass
    import concourse.tile as tile
    from concourse import bacc, mybir

    f32 = mybir.dt.float32
    DT = {"fp32": mybir.dt.float32, "bf16": mybir.dt.bfloat16,
          "fp32r": mybir.dt.float32r}[mmdt]
    AF = mybir.ActivationFunctionType

    ND_WARM = int(os.environ.get("BASS_KERNEL_NDWARM", "0"))

    nc = bacc.Bacc("TRN2", target_bir_lowering=False, debug=False,
                   num_devices=N_CORES)

    def din(name, shape, dt=None):
        return nc.dram_tensor(name, list(shape), dt or f32,
                              kind="ExternalInput")

    # weights; gate order inside dim-1 is (i, f, o, g)
    whh0 = din("whh0", (128, 4, KT, 128), DT)
    wih1 = din("wih1", (128, 4, KT, 128), DT)
    whh1 = din("whh1", (128, 4, KT, 128), DT)
    wo1 = din("wo1", (128, 4, KT, 128), DT)
    wih0 = din("wih0", (9, 4, 128), DT)      # row 8 = gate bias (ones trick)
    wo2 = din("wo2", (128, 4, 8), DT)
    wproj = din("wproj", (128, 9, 2, 128))   # col TOT = b_proj (ones trick)
    zt = din("zt", (128, 2, B))
    bo2 = din("bo2", (8, 1))
    xinit = din("xinit", (9, NB), DT)        # zeros + ones row 8
    if has_bias1:
        bias1 = din("bias1", (128, 4, NB))   # broadcast of b_ih1+b_hh1
    if has_bo1:
        bo1b = din("bo1b", (128, 4, NB))     # broadcast of b_o1

    OUT = nc.dram_tensor("out", [seq, 8, B], f32, kind="ExternalOutput")

    def persist(name, shape, dtype=f32):
        return nc.alloc_sbuf_tensor(name, list(shape), dtype).ap()

    whh0_sb = persist("whh0_sb", [128, 4, KT, 128], DT)
    wih1_sb = persist("wih1_sb", [128, 4, KT, 128], DT)
    whh1_sb = persist("whh1_sb", [128, 4, KT, 128], DT)
    wo1_sb = persist("wo1_sb", [128, 4, KT, 128], DT)
    wih0_sb = persist("wih0_sb", [9, 4, 128], DT)
    wo2_sb = persist("wo2_sb", [128, 4, 8], DT)
    wproj_sb = persist("wproj_sb", [128, 9, 2, 128])
    zt_sb = persist("zt_sb", [128, 2, B])
    bo2_sb = persist("bo2_sb", [8, 1])
    if has_bias1:
        bias1_sb = persist("bias1_sb", [128, 4, NB])
    if has_bo1:
        bo1b_sb = persist("bo1b_sb", [128, 4, NB])

    class Chain:
        def __init__(self, name, b0):
            self.name = name
            self.b0 = b0                      # batch offset into OUT
            self.h1T = persist(f"h1T_{name}", [128, KT, NB], DT)
            self.h2T = persist(f"h2T_{name}", [128, KT, NB], DT)
            self.c1 = persist(f"c1_{name}", [128, NB])
            self.c2 = persist(f"c2_{name}", [128, NB])
            self.xT = persist(f"xT_{name}", [9, NB], DT)
            self.g0 = None                    # open gates0 psum bank
            self.g1 = None                    # open gates1 psum bank

    with tile.TileContext(nc) as tc:
        A = Chain("a", 0)
        Bc = Chain("b", NB)
        chains = (A, Bc)

        loads = [
            (whh0_sb, whh0), (wih1_sb, wih1), (whh1_sb, whh1), (wo1_sb, wo1),
            (wih0_sb, wih0), (wo2_sb, wo2), (wproj_sb, wproj), (zt_sb, zt),
            (bo2_sb, bo2),
        ]
        if has_bias1:
            loads.append((bias1_sb, bias1))
        if has_bo1:
            loads.append((bo1b_sb, bo1b))
        for dst, src in loads:
            nc.sync.dma_start(dst[:], src.ap())
        for ch in chains:
            nc.scalar.dma_start(ch.xT[:], xinit.ap())

        G_TAGS = ["g1a", "g1b", "g0a", "g0b", "ma", "mb"]

        with (
            tc.tile_pool(name="ps", bufs=1, space="PSUM") as ps,
            tc.tile_pool(name="psx", bufs=2, space="PSUM") as psx,
            tc.tile_pool(name="nl", bufs=2) as nl,
            tc.tile_pool(name="dram", bufs=2, space="DRAM") as dram,
        ):
            # ---- init: h0 full + own c0 chunk via W_proj (both chains) ----
            for m in range(9):
                pst = ps.tile([128, B], f32, name="ps_init",
                              tag=G_TAGS[m % 6])
                nc.tensor.matmul(pst[:], wproj_sb[:, m, 0, :], zt_sb[:, 0, :],
                                 start=True, stop=False)
                nc.tensor.matmul(pst[:], wproj_sb[:, m, 1, :], zt_sb[:, 1, :],
                                 start=False, stop=True)
                for ch in chains:
                    sl = pst[:, ch.b0:ch.b0 + NB]
                    if m < 8:
                        nc.vector.tensor_copy(ch.h1T[:, m, :], sl)
                        nc.scalar.activation(ch.h2T[:, m, :], sl, AF.Identity)
                    else:
                        nc.vector.tensor_copy(ch.c1[:, :], sl)
                        nc.scalar.activation(ch.c2[:, :], sl, AF.Identity)

            def warm_pe(nwarm):
                if nwarm <= 0:
                    return
                dps = psx.tile([128, NB], f32, name="ps_warm", tag="x")
                for i in range(nwarm):
                    nc.tensor.matmul(dps[:], whh1_sb[:, 0, i % KT, :],
                                     whh1_sb[:, 1, i % KT, :],
                                     start=True, stop=True)

            def lstm_nonlin(ch, G, c_sb, lay, bias_sb):
                """G: psum bank [128, 4, NB], gates (i, f, o, g)."""
                sfx = f"{ch.name}{lay}"
                if bias_sb is not None:
                    nc.vector.tensor_add(G[:, :, :], G[:, :, :], bias_sb[:])
                sig = nl.tile([128, 3, NB], f32, name="sig", tag=f"s_{sfx}")
                tan_g = nl.tile([128, NB], f32, name="tan_g", tag=f"tg_{sfx}")
                nc.scalar.activation(sig[:], G[:, 0:3, :], AF.Sigmoid)
                nc.scalar.activation(tan_g[:], G[:, 3, :], AF.Tanh)
                t_fc = nl.tile([128, NB], f32, name="t_fc", tag=f"fc_{sfx}")
                t_ig = nl.tile([128, NB], f32, name="t_ig", tag=f"ig_{sfx}")
                nc.vector.tensor_mul(t_fc[:], sig[:, 1, :], c_sb[:, :])
                nc.vector.tensor_mul(t_ig[:], sig[:, 0, :], tan_g[:])
                nc.vector.tensor_add(c_sb[:, :], t_fc[:], t_ig[:])
                tan_c = nl.tile([128, NB], f32, name="tan_c", tag=f"tc_{sfx}")
                nc.scalar.activation(tan_c[:], c_sb[:, :], AF.Tanh)
                hch = nl.tile([128, NB], DT, name="hch", tag=f"h_{sfx}")
                nc.vector.tensor_mul(hch[:], sig[:, 2, :], tan_c[:])
                return hch

            def gather(ch, hch, dest, lay):
                sfx = f"{ch.name}{lay}"
                inb = dram.tile([128, NB], DT, name="agin", tag=f"agi_{sfx}")
                outb = dram.tile([128 * N_CORES, NB], DT, name="agout",
                                 tag=f"ago_{sfx}", addr_space="Shared")
                nc.scalar.dma_start(inb[:], hch[:])
                nc.gpsimd.collective_compute(
                    "AllGather", mybir.AluOpType.bypass,
                    replica_groups=[list(range(N_CORES))],
                    ins=[inb.opt()], outs=[outb.opt()],
                )
                for eng, k0, nk in ((nc.sync, 0, 2), (nc.scalar, 2, 2),
                                    (nc.sync, 4, 2), (nc.scalar, 6, 2)):
                    eng.dma_start(
                        dest[:, k0:k0 + nk, :],
                        outb[k0 * 128:(k0 + nk) * 128, :].rearrange(
                            "(k p) n -> p k n", p=128))

            def emit_gates0_tail(ch):
                """wih0 into the open G0 bank, nonlin0, launch AG1."""
                G0 = ch.g0
                for g in range(4):
                    nc.tensor.matmul(G0[:, g, :], wih0_sb[:, g, :],
                                     ch.xT[:, :], start=False, stop=(g == 3))
                h1ch = lstm_nonlin(ch, G0, ch.c1, 0, None)
                gather(ch, h1ch, ch.h1T, 0)

            def emit_whh0(ch):
                """Whh0 @ h1 (ready work) -> open G0 bank [128, 4, NB]."""
                G0 = ps.tile([128, 4, NB], f32, name="ps_g0",
                             tag=f"g0{ch.name}")
                for k in range(KT):
                    for g in range(4):
                        nc.tensor.matmul(G0[:, g, :], whh0_sb[:, g, k, :],
                                         ch.h1T[:, k, :],
                                         start=(k == 0 and g == 0),
                                         stop=False)
                ch.g0 = G0

            def emit_front(ch):
                """Whh1 (ready work), then Wih1 (stalls on AG1), then the
                layer-1 nonlinearity and the h2 AllGather launch."""
                G1 = ps.tile([128, 4, NB], f32, name="ps_g1",
                             tag=f"g1{ch.name}")
                for k in range(KT):
                    for g in range(4):
                        nc.tensor.matmul(G1[:, g, :], whh1_sb[:, g, k, :],
                                         ch.h2T[:, k, :],
                                         start=(k == 0 and g == 0),
                                         stop=False)
                warm_pe(ND_WARM)
                for k in range(KT):
                    for g in range(4):
                        nc.tensor.matmul(G1[:, g, :], wih1_sb[:, g, k, :],
                                         ch.h1T[:, k, :],
                                         start=False,
                                         stop=(k == KT - 1 and g == 3))
                h2ch = lstm_nonlin(ch, G1, ch.c2, 1,
                                   bias1_sb[:] if has_bias1 else None)
                gather(ch, h2ch, ch.h2T, 1)

            def emit_mlp(ch, t, last):
                """MLP(t) (stalls on AG2), OUT write, then xT cast; if not
                last, wih0 completes gates0 and nonlin0 + AG1 launch."""
                warm_pe(ND_WARM)
                M = ps.tile([128, 4, NB], f32, name="ps_m",
                            tag=f"m{ch.name}")
                for k in range(KT):
                    for m in range(4):
                        nc.tensor.matmul(M[:, m, :], wo1_sb[:, m, k, :],
                                         ch.h2T[:, k, :],
                                         start=(k == 0 and m == 0),
                                         stop=(k == KT - 1 and m == 3))
                relu = nl.tile([128, 4, NB], DT, name="relu",
                               tag=f"relu_{ch.name}")
                if has_bo1:
                    nc.vector.tensor_add(M[:, :, :], M[:, :, :], bo1b_sb[:])
                nc.scalar.activation(relu[:], M[:], AF.Relu)
                ps_x = psx.tile([8, NB], f32, name="ps_x", tag="x")
                for k in range(4):
                    nc.tensor.matmul(ps_x[:], wo2_sb[:, k, :], relu[:, k, :],
                                     start=(k == 0), stop=(k == 3))
                if has_bo2:
                    nc.scalar.activation(ch.xT[:8, :], ps_x[:], AF.Identity,
                                         bias=bo2_sb[:, 0:1])
                else:
                    nc.vector.tensor_copy(ch.xT[:8, :], ps_x[:])
                xTf = nl.tile([8, NB], f32, name="xTf", tag=f"xf_{ch.name}")
                nc.scalar.activation(xTf[:], ps_x[:], AF.Identity,
                                     bias=bo2_sb[:, 0:1])
                nc.sync.dma_start(OUT.ap()[t][:, ch.b0:ch.b0 + NB], xTf[:])
                if not last:
                    emit_gates0_tail(ch)

            # ---- step 0 prologue: gates0(0) = Whh0@h0 + Wih0@[0;1] ----
            for ch in chains:
                emit_whh0(ch)
                emit_gates0_tail(ch)

            for t in range(seq):
                last = t == seq - 1
                for ch in chains:
                    emit_front(ch)
                if not last:
                    for ch in chains:
                        emit_whh0(ch)
                for ch in chains:
                    emit_mlp(ch, t, last)

    nc.compile()
    return nc


def _lhsT_tiles(W, rows, K):
    """W[rows] viewed as lhsT tiles: [128, MT, KTl, 128] with
    out[ki, mt, kt, mi] = W[rows[mt*128+mi], kt*128+ki]."""
    R = len(rows)
    MT = R // 128
    KTl = K // 128
    t = W[rows].reshape(MT, 128, KTl, 128)          # [mt, mi, kt, ki]
    return np.ascontiguousarray(t.transpose(3, 0, 2, 1)).astype(np.float32)


GATE_PERM = (0, 1, 3, 2)  # pytorch (i,f,g,o) -> kernel (i,f,o,g)


def _prep_inputs(inputs):
    import ml_dtypes
    np_dt = {"fp32": np.float32, "bf16": ml_dtypes.bfloat16,
             "fp32r": np.float32}[_mmdt()]
    f = lambda k: np.asarray(inputs[k], np.float32)
    W_proj, b_proj = f("W_proj"), f("b_proj")
    W_ih0, W_hh0 = f("W_ih0"), f("W_hh0")
    b_ih0, b_hh0 = f("b_ih0"), f("b_hh0")
    W_ih1, W_hh1 = f("W_ih1"), f("W_hh1")
    b_ih1, b_hh1 = f("b_ih1"), f("b_hh1")
    W_o1, b_o1 = f("W_o1"), f("b_o1")
    W_o2, b_o2 = f("W_o2"), f("b_o2")
    z = np.concatenate([f("z_primitive"), f("z_skill"), f("z_style")], axis=1)

    wo1 = _lhsT_tiles(W_o1, np.arange(512), H)
    wo2 = np.ascontiguousarray(
        W_o2.T.reshape(4, 128, 8).transpose(1, 0, 2)).astype(np.float32)
    bo2 = b_o2.reshape(8, 1).astype(np.float32)
    bias_g1 = b_ih1 + b_hh1
    has_bias1 = bool(np.any(bias_g1 != 0.0))
    has_bo1 = bool(np.any(b_o1 != 0.0))
    has_bo2 = bool(np.any(b_o2 != 0.0))

    ztp = np.zeros((256, B), np.float32)
    ztp[:TOT] = z.T
    ztp[TOT] = 1.0
    zt = np.ascontiguousarray(ztp.reshape(2, 128, B).transpose(1, 0, 2))
    Wp = np.zeros((2 * H, 256), np.float32)
    Wp[:, :TOT] = W_proj
    Wp[:, TOT] = b_proj
    bias_g0 = b_ih0 + b_hh0
    xinit = np.zeros((9, NB), np.float32)
    xinit[8] = 1.0

    in_maps = []
    for c in range(N_CORES):
        rows_g = np.concatenate(
            [GATE_PERM[g] * H + c * HC + np.arange(HC) for g in range(4)])
        rows_p = np.concatenate([np.arange(H), H + c * HC + np.arange(HC)])
        wih0 = np.zeros((9, 4, 128), np.float32)
        wih0[:8] = W_ih0[rows_g].reshape(4, 128, 8).transpose(2, 0, 1)
        wih0[8] = bias_g0[rows_g].reshape(4, 128)
        im = {
            "whh0": _lhsT_tiles(W_hh0, rows_g, H).astype(np_dt),
            "wih1": _lhsT_tiles(W_ih1, rows_g, H).astype(np_dt),
            "whh1": _lhsT_tiles(W_hh1, rows_g, H).astype(np_dt),
            "wo1": wo1.astype(np_dt),
            "wih0": wih0.astype(np_dt),
            "wo2": wo2.astype(np_dt),
            "wproj": _lhsT_tiles(Wp, rows_p, 256),
            "zt": zt,
            "bo2": bo2,
            "xinit": xinit.astype(np_dt),
        }
        if has_bias1:
            im["bias1"] = np.broadcast_to(
                bias_g1[rows_g].reshape(4, 128).T[:, :, None],
                (128, 4, NB)).astype(np.float32).copy()
        if has_bo1:
            im["bo1b"] = np.broadcast_to(
                b_o1.reshape(4, 128).T[:, :, None],
                (128, 4, NB)).astype(np.float32).copy()
        in_maps.append(im)
    return in_maps, has_bias1, has_bo1, has_bo2


def kernel(**inputs):
    from concourse.bass_utils import run_bass_kernel_spmd

    seq = int(os.environ.get("BASS_KERNEL_SEQ", SEQ))
    in_maps, has_bias1, has_bo1, has_bo2 = _prep_inputs(inputs)
    key = (seq, _mmdt(), has_bias1, has_bo1, has_bo2)
    if key not in _CACHE:
        _CACHE[key] = _build(seq, _mmdt(), has_bias1, has_bo1, has_bo2)
    nc = _CACHE[key]

    trace = os.environ.get("BASS_KERNEL_TRACE", "") == "1"
    kwargs = {}
    if trace:
        kwargs["trace"] = True
        kwargs["tmpdir"] = os.environ.get("BASS_KERNEL_TRACE_DIR") or None
    res = run_bass_kernel_spmd(nc, in_maps, core_ids=list(range(N_CORES)),
                               **kwargs)
    if trace:
        kernel.last_exec_time_ns = res.exec_time_ns
    out = res.results[0]["out"]          # [seq, 8, B]
    return np.ascontiguousarray(out.transpose(2, 0, 1)).astype(np.float32)


kernel.last_exec_time_ns = None
